# revision 21
# baseline (speedup 1.0000x reference)
"""Blended-MoE 3-layer MLP (nn_Expert) on 8 Trainium2 NeuronCores.

Math: per layer, y[b,o] = act( sum_e blend[b,e] * (W[e] @ x[b] + B[e])[o] ).
Rewritten as a dense matmul with a per-expert prescale of the activations:
  y[o,b] = act( sum_e sum_k Wf[e*I+k, o] * (blend[e,b] * hT[k,b]) + bias )
where Wf[(e,i), o] = W[e,o,i] and everything is kept transposed on-chip
([feature, batch] layout) so each layer's output feeds the next directly.

Sharding: data-parallel over the batch — 4096 tokens -> 512 per core; the
per-expert weight stacks are replicated. Matmuls run in bf16 (full PE rate,
~5e-3 rel err). PSUM accumulates over experts and the contraction in fp32.

Perf structure:
- All tensors bf16 on the wire and in SBUF (activations included); PSUM and
  the ELU drain stay fp32.
- Weights are host-packed per (expert, k-group) into one contiguous
  [128, GW*width] block so each group is 1-2 DMA dispatches (dispatch cost
  on the Sync engine is ~600ns each; fewer dispatches >> smaller ones).
- The per-expert blend broadcast (brep) is generated on-chip with K=1
  matmuls (ones.T @ blT-row) instead of a 2MB HBM load; those matmuls
  double as HAM warmup so the PE clock is at 2.4GHz when the real stream
  starts.
- The first weight groups and the first x tile are dispatched before
  anything else, 4-way split across DMA queues, so the PE never starves at
  startup (a starved PE re-triggers the HAM throttle to half clock).
- The last TWO k-groups of every output half run bank(j)-outer so PSUM
  banks stop staggered ~1.7us apart and their ELU/output drains overlap
  the compute tail instead of serializing after it.
"""

import os

import numpy as np
import ml_dtypes

import concourse.bass as bass
import concourse.tile as tile
import concourse.mybir as mybir
from concourse import bacc
from concourse.bass_utils import run_bass_kernel_spmd
from contextlib import ExitStack

dt = mybir.dt
ALU = mybir.AluOpType
ACTF = mybir.ActivationFunctionType

N_CORES = 8
B_FULL = 4096
BC = B_FULL // N_CORES  # 512 tokens per core
E = 8
DIMS = [1024, 2048, 2048, 512]
LAYERS = [  # (I, O, has_elu)
    (1024, 2048, True),
    (2048, 2048, True),
    (2048, 512, False),
]
OH = 1024   # o-columns per half-pass (8 psum banks)
GW = 4      # k-blocks packed per wide activation tile / weight group
WIDE = GW * BC

MM_DT = dt.bfloat16
MM_NP = ml_dtypes.bfloat16

_cache = {}


def _build(with_bias=True):
    nc = bacc.Bacc("TRN2", target_bir_lowering=False, debug=False,
                   num_devices=N_CORES)
    NG0 = DIMS[0] // (128 * GW)
    xTw = nc.declare_dram_parameter("xTw", [NG0, 128, WIDE], MM_DT,
                                    isOutput=False)
    blT = nc.declare_dram_parameter("blT", [E, BC], MM_DT, isOutput=False)
    blF = nc.declare_dram_parameter("blF", [1, E * BC], MM_DT, isOutput=False)
    # weights pre-packed per (o-half, expert*k-group) as [128, GW*width] so
    # each group is one contiguous 0.5-1MB DRAM block
    wg = []
    for l, (I, O, _) in enumerate(LAYERS):
        NG = I // (128 * GW)
        nh = max(O // OH, 1)
        width = min(OH, O)
        wg.append(nc.declare_dram_parameter(
            f"w{l}g", [nh, E * NG, 128, GW * width], MM_DT, isOutput=False))
    bf = [nc.declare_dram_parameter(f"b{l}f", [E, O], MM_DT, isOutput=False)
          for l, (I, O, _) in enumerate(LAYERS)]
    yT = nc.declare_dram_parameter("yT", [DIMS[3], BC], dt.float32,
                                   isOutput=True)

    tc = tile.TileContext(nc)
    with tc:
        with ExitStack() as ctx:
            const = ctx.enter_context(tc.tile_pool(name="const", bufs=1))
            act = ctx.enter_context(tc.tile_pool(name="act", bufs=1))
            xpool = ctx.enter_context(tc.tile_pool(name="xpool", bufs=3))
            wpool = ctx.enter_context(tc.tile_pool(name="wpool", bufs=8))
            tpool = ctx.enter_context(tc.tile_pool(name="tpool", bufs=2))
            ypool = ctx.enter_context(tc.tile_pool(name="ypool", bufs=2))
            pp = ctx.enter_context(tc.tile_pool(name="pp", bufs=8,
                                                space="PSUM"))

            # --- constants (no DMA deps) ---
            ones_f = const.tile([1, 128], dt.float32, tag="ones_f")
            nc.vector.memset(ones_f[:], 1.0)
            ones = const.tile([1, 128], MM_DT, tag="ones")
            nc.vector.tensor_copy(ones[:], ones_f[:])
            wsrc_f = const.tile([1, BC], dt.float32, tag="wsrc_f")
            nc.vector.memset(wsrc_f[:], 1.0)
            wsrc = const.tile([1, BC], MM_DT, tag="wsrc")
            nc.vector.tensor_copy(wsrc[:], wsrc_f[:])

            # --- early DMA dispatches across THREE parallel queue paths:
            # weights split sync(SP-HWDGE)/scalar(ACT-HWDGE) halves, x tiles
            # on the gpsimd software-DGE queue, so the first matmul wave
            # only waits ~2-3us instead of serializing 4.5MB on one queue ---
            blF_sb = const.tile([1, E * BC], MM_DT, tag="blF")
            nc.sync.dma_start(blF_sb[:], blF[:])
            blT_sb = None
            if with_bias:
                blT_sb = const.tile([E, BC], MM_DT, tag="blT")
                nc.sync.dma_start(blT_sb[:], blT[:])

            W0 = LAYERS[0][1]
            width0 = min(OH, W0)
            h0 = [act.tile([128, WIDE], MM_DT, name=f"h0_{g}", tag=f"h0_{g}")
                  for g in range(NG0)]
            preload = {}

            def wg_tile(l, half, grp, width):
                t = wpool.tile([128, GW * width], MM_DT, tag="wg")
                hw = GW * width // 2
                nc.sync.dma_start(t[:, :hw], wg[l][half, grp, :, :hw])
                nc.scalar.dma_start(t[:, hw:], wg[l][half, grp, :, hw:])
                return t

            # first group quarter-split + x chunks, interleaved across the
            # two HWDGE queues in first-use order (gpsimd's software queue
            # starts ~10us late — don't put startup-critical loads there)
            t = wpool.tile([128, GW * width0], MM_DT, tag="wg")

            def boot_q(c, eng):
                eng.dma_start(t[:, c * width0:(c + 1) * width0],
                              wg[0][0, 0, :, c * width0:(c + 1) * width0])

            def boot_x(g, c, eng):
                eng.dma_start(h0[g][:, c * BC:(c + 1) * BC],
                              xTw[g, :, c * BC:(c + 1) * BC])

            boot_x(0, 0, nc.sync)
            boot_x(0, 2, nc.scalar)
            boot_q(0, nc.sync)
            boot_q(2, nc.scalar)
            boot_x(0, 1, nc.sync)
            boot_x(0, 3, nc.scalar)
            boot_q(1, nc.sync)
            boot_q(3, nc.scalar)
            preload[(0, 0, 0)] = t
            preload[(0, 0, 1)] = wg_tile(0, 0, 1, width0)
            boot_x(1, 0, nc.sync)
            boot_x(1, 2, nc.scalar)
            boot_x(1, 1, nc.sync)
            boot_x(1, 3, nc.scalar)
            for grp in range(2, 6):
                preload[(0, 0, grp)] = wg_tile(0, 0, grp, width0)

            # --- HAM warmup + on-chip brep generation ---
            # 3 throwaway K=1 matmuls (no DMA deps) start the PE activity
            # window while blF lands, then 8 useful K=1 matmuls broadcast
            # each blend row into a PSUM bank. The PSUM->SBUF copies all go
            # on the DVE (the ACT queue carries weight-DMA dispatches and
            # would head-of-line-block them); copy 0 now, 1..7 interleaved
            # into the boot block below so the first prescale isn't delayed.
            brep_sb = [None] * E
            brep_ps = [None] * E
            warm = pp.tile([128, BC], dt.float32, tag="ps")
            for _ in range(3):
                nc.tensor.matmul(warm[:], ones[:], wsrc[:],
                                 start=True, stop=True)
            for e in range(E):
                ps_e = pp.tile([128, BC], dt.float32, tag="ps")
                nc.tensor.matmul(ps_e[:], ones[:],
                                 blF_sb[:, e * BC:(e + 1) * BC],
                                 start=True, stop=True)
                brep_ps[e] = ps_e
            # filler matmuls bridge the PE-idle window until the first
            # weight tile's completion semaphore (~3.5us) so the HAM clock
            # stays at 2.4GHz when the real stream starts
            warm2 = pp.tile([128, BC], dt.float32, tag="ps")
            for _ in range(8):
                nc.tensor.matmul(warm2[:], ones[:], wsrc[:],
                                 start=True, stop=True)

            def brep_cp(e):
                t = const.tile([128, BC], MM_DT, tag=f"brep{e}")
                nc.vector.tensor_copy(t[:], brep_ps[e][:])
                brep_sb[e] = t

            brep_cp(0)

            bf_sb = [None, None, None]
            if with_bias:
                for l, (I, O, _) in enumerate(LAYERS):
                    t = const.tile([E, O], MM_DT, tag=f"bf{l}")
                    nc.sync.dma_start(t[:], bf[l][:])
                    bf_sb[l] = t

            hT = h0
            pending_drain = []  # deferred drain emission from previous half

            for l, (I, O, has_elu) in enumerate(LAYERS):
                NG = I // (128 * GW)
                width = min(OH, O)
                h_next = []
                if l < len(LAYERS) - 1:
                    for g in range(O * BC // (128 * WIDE)):
                        h_next.append(act.tile([128, WIDE], MM_DT,
                                               name=f"h{l + 1}_{g}",
                                               tag=f"h{l + 1}_{g}"))
                for half in range(max(O // OH, 1)):
                    half_start = half * OH
                    n_ot = width // 128
                    ps = []
                    for j in range(n_ot):
                        p = pp.tile([128, BC], dt.float32, tag="ps")
                        if with_bias:
                            nc.tensor.matmul(
                                p[:],
                                bf_sb[l][:, half_start + j * 128:
                                         half_start + (j + 1) * 128],
                                blT_sb[:],
                                start=True, stop=False)
                        ps.append(p)
                    tail_pairs = []  # (xp, wt) of the last two k-groups
                    for e in range(E):
                        for g in range(NG):
                            grp = e * NG + g
                            wt = preload.pop((l, half, grp), None)
                            if wt is None:
                                wt = wg_tile(l, half, grp, width)
                            xp = xpool.tile([128, WIDE], MM_DT, tag="xp")
                            if l == 0 and half == 0 and e == 0 and g == 0:
                                # boot: per-chunk prescale + matmuls so the
                                # first wave only waits for chunk 0 of x;
                                # remaining brep copies right after the
                                # first prescale (they gate PSUM bank reuse
                                # and the later experts' prescales)
                                for c in range(GW):
                                    nc.vector.tensor_tensor(
                                        xp[:, c * BC:(c + 1) * BC],
                                        hT[g][:, c * BC:(c + 1) * BC],
                                        brep_sb[e][:], ALU.mult)
                                    for j in range(n_ot):
                                        nc.tensor.matmul(
                                            ps[j][:],
                                            wt[:, c * width + j * 128:
                                               c * width + (j + 1) * 128],
                                            xp[:, c * BC:(c + 1) * BC],
                                            start=(not with_bias and c == 0),
                                            stop=False)
                                    if c == 0:
                                        for e2 in range(1, E):
                                            brep_cp(e2)
                                continue
                            nc.vector.tensor_tensor(
                                xp[:].rearrange("p (c b) -> p c b", c=GW),
                                hT[g][:].rearrange("p (c b) -> p c b", c=GW),
                                brep_sb[e][:].unsqueeze(1).broadcast_to(
                                    (128, GW, BC)),
                                ALU.mult)
                            if e == E - 1 and g >= NG - 2:
                                # last two groups: bank-outer below
                                tail_pairs.append((xp, wt))
                                if g < NG - 1:
                                    continue
                                for j in range(n_ot):
                                    for gi, (xp2, wt2) in enumerate(tail_pairs):
                                        for c in range(GW):
                                            nc.tensor.matmul(
                                                ps[j][:],
                                                wt2[:, c * width + j * 128:
                                                    c * width + (j + 1) * 128],
                                                xp2[:, c * BC:(c + 1) * BC],
                                                start=False,
                                                stop=(gi == len(tail_pairs) - 1
                                                      and c == GW - 1))
                                continue
                            for c in range(GW):
                                opener = (not with_bias and e == 0 and g == 0
                                          and c == 0)
                                for j in range(n_ot):
                                    nc.tensor.matmul(
                                        ps[j][:],
                                        wt[:, c * width + j * 128:
                                           c * width + (j + 1) * 128],
                                        xp[:, c * BC:(c + 1) * BC],
                                        start=opener, stop=False)
                            if e == 0 and g == 0 and pending_drain:
                                # emit the previous half's drains only after
                                # this half's first wave, so the scheduler
                                # prioritizes restarting the PE pipeline
                                for fn in pending_drain:
                                    fn()
                                pending_drain = []

                    def make_drain(l, has_elu, half_start, n_ot, ps, h_next):
                        def drain():
                            for j in range(n_ot):
                                ot = (half_start + j * 128) // 128
                                if has_elu:
                                    # elu(v) = relu(v) + exp(min(v,0)) - 1
                                    m = tpool.tile([128, BC], dt.float32,
                                                   tag="m")
                                    nc.vector.tensor_scalar_min(
                                        m[:], ps[j][:], 0.0)
                                    r = tpool.tile([128, BC], dt.float32,
                                                   tag="r")
                                    nc.scalar.activation(r[:], ps[j][:],
                                                         ACTF.Relu)
                                    x2 = tpool.tile([128, BC], dt.float32,
                                                    tag="x2")
                                    nc.scalar.activation(x2[:], m[:], ACTF.Exp)
                                    dst = h_next[ot // GW][
                                        :, (ot % GW) * BC:(ot % GW + 1) * BC]
                                    nc.vector.scalar_tensor_tensor(
                                        dst, x2[:], -1.0, r[:],
                                        ALU.add, ALU.add)
                                else:
                                    # half-split copies (alternating DVE/ACT)
                                    # so each output DMA overlaps the next
                                    # copy; halves go to both HWDGE queues
                                    y = ypool.tile([128, BC], dt.float32,
                                                   tag="y")
                                    rows = slice(half_start + j * 128,
                                                 half_start + (j + 1) * 128)
                                    hb = BC // 2
                                    for hf in range(2):
                                        cs = slice(hf * hb, (hf + 1) * hb)
                                        if (2 * j + hf) % 2 == 0:
                                            nc.vector.tensor_copy(
                                                y[:, cs], ps[j][:, cs])
                                        else:
                                            nc.scalar.activation(
                                                y[:, cs], ps[j][:, cs],
                                                ACTF.Identity)
                                        eng = nc.sync if hf == 0 else nc.scalar
                                        eng.dma_start(yT[rows, cs], y[:, cs])
                        return drain
                    pending_drain.append(
                        make_drain(l, has_elu, half_start, n_ot, ps, h_next))
                hT = h_next
            for fn in pending_drain:
                fn()
    nc.compile()
    return nc


def _prep_inputs(weight_blend, x, W0, B0, W1, B1, W2, B2):
    Ws = [W0, W1, W2]
    Bs = [B0, B1, B2]
    shared = {}
    for l in range(3):
        I, O, _ = LAYERS[l]
        NG = I // (128 * GW)
        nh = max(O // OH, 1)
        width = min(OH, O)
        wfl = np.asarray(Ws[l]).transpose(0, 2, 1).reshape(E * I, O)
        halves = []
        for h in range(nh):
            cols = wfl[:, h * OH:h * OH + width]          # [E*I, width]
            grpd = (cols.reshape(E * NG, GW, 128, width)
                        .transpose(0, 2, 1, 3)
                        .reshape(E * NG, 128, GW * width))
            halves.append(grpd)
        shared[f"w{l}g"] = np.ascontiguousarray(np.stack(halves), dtype=MM_NP)
        shared[f"b{l}f"] = np.ascontiguousarray(
            np.asarray(Bs[l])[:, :, 0], dtype=MM_NP)
    in_maps = []
    for c in range(N_CORES):
        s = slice(c * BC, (c + 1) * BC)
        blT = np.ascontiguousarray(np.asarray(weight_blend)[s].T,
                                   dtype=np.float32)
        m = dict(shared)
        # pack x.T into [NG, 128, GW*BC] wide tiles: block kt = g*GW + c
        xt = np.ascontiguousarray(np.asarray(x)[s].T)     # [1024, 512]
        m["xTw"] = np.ascontiguousarray(
            xt.reshape(-1, GW, 128, BC).transpose(0, 2, 1, 3)
              .reshape(-1, 128, WIDE), dtype=MM_NP)
        m["blT"] = blT.astype(MM_NP)
        m["blF"] = np.ascontiguousarray(
            blT.astype(MM_NP).reshape(1, -1))
        in_maps.append(m)
    return in_maps


def run(inputs, trace=False, tmpdir=None, trace_cores=None):
    """Run on hardware; returns (y, BassKernelResults)."""
    with_bias = any(
        np.any(np.asarray(inputs[k])) for k in ("B0", "B1", "B2"))
    key = ("nc", with_bias)
    if key not in _cache:
        _cache[key] = _build(with_bias)
    nc = _cache[key]
    in_maps = _prep_inputs(**inputs)
    kw = {}
    if tmpdir:
        kw["tmpdir"] = tmpdir
    if trace_cores:
        kw["trace_cores"] = trace_cores
    res = run_bass_kernel_spmd(
        nc, in_maps, core_ids=list(range(N_CORES)), trace=trace, **kw)
    y = np.concatenate([r["yT"].T for r in res.results], axis=0)
    return np.ascontiguousarray(y, dtype=np.float32), res


def kernel(**inputs):
    y, _ = run(inputs, trace=False)
    return y
